# revision 19
# baseline (speedup 1.0000x reference)
"""EquiFormer encoder (l=0 collapsed) on 8 TRN2 NeuronCores, data-parallel over batch.

The reference's spherical-harmonic rows 1..8 stay exactly zero through the network
(zero init; every per-m op maps 0->0), so only the l=0 channel is computed on device.
Output rows 1..8 are exact zeros, assembled on host.
"""
import sys

sys.path.insert(0, "/opt/trn_rl_repo")
import numpy as np
import concourse.bass as bass
import concourse.bacc as bacc
import concourse.mybir as mybir
import concourse.tile as tile
from concourse.bass_utils import run_bass_kernel_spmd

AF = mybir.ActivationFunctionType
ALU = mybir.AluOpType
F32 = mybir.dt.float32
U16 = mybir.dt.uint16

RADII = [0.05, 0.2, 0.8, 3.0]
K = 16
NB = 64
H = 4
CIN = [32, 64, 128, 128]
COUT = [64, 128, 128, 64]
EC = [16, 32, 64, 128]
CV = [4, 8, 16, 32]
FF = [32, 64, 128, 256]
NS = [4096, 1024, 256, 64]
ND = [1024, 256, 64, 1]
B_TOTAL, N0 = 16, 4096
N_CORES = 8
B_PER = B_TOTAL // N_CORES
PI = float(np.pi)
SIM_COMPAT = False  # sim_test sets True: CoreSim lacks Silu
DEBUG = False
TRACE = False
LAST_RESULTS = None


def _silu(nc, wpool, out_sb, in_ps, bias_ap, rows, cols):
    if not SIM_COMPAT:
        if bias_ap is None:
            nc.scalar.activation(out_sb, in_ps, AF.Silu)
        else:
            nc.scalar.activation(out_sb, in_ps, AF.Silu, bias=bias_ap)
        return
    tmp = wpool.tile([rows, cols], F32, tag="silu_tmp")
    if bias_ap is None:
        nc.scalar.activation(out_sb, in_ps, AF.Sigmoid)
        nc.vector.tensor_copy(tmp[0:rows, 0:cols], in_ps)
    else:
        nc.scalar.activation(out_sb, in_ps, AF.Sigmoid, bias=bias_ap)
        nc.vector.tensor_scalar_add(tmp[0:rows, 0:cols], in_ps, bias_ap)
    nc.vector.tensor_tensor(out=out_sb, in0=out_sb, in1=tmp[0:rows, 0:cols],
                            op=ALU.mult)


def cfg(n):
    cutoff = RADII[n] * 0.99
    hcv = H * CV[n]
    vb = hcv // 16
    s0base = 32 if n == 0 else 16 * vb       # 32-aligned [s0l|coords] block base
    le_off = 32 * ((hcv + 31) // 32)         # 32-aligned logitsE rows in gl psum
    return dict(
        n=n, ns=NS[n], nd=ND[n], cin=CIN[n], cout=COUT[n], ec=EC[n],
        hcv=hcv, cv=CV[n], ff=FF[n], cutoff=cutoff, std=cutoff / NB,
        e=ND[n] * K, ntiles=max(1, ND[n] // 128), tr=min(128, ND[n]),
        vb=vb, s0base=s0base, le_off=le_off,
        wa=hcv + 16, wb=hcv + 4, stride=4 ** n,
    )


def _mm(nc, out_ap, lhsT, rhs_ap, n_cols, start=True, stop=True):
    for cc in range(0, n_cols, 512):
        nw = min(512, n_cols - cc)
        nc.tensor.matmul(out_ap[:, cc:cc + nw], lhsT, rhs_ap[:, cc:cc + nw],
                         start=start, stop=stop)


def build_nc():
    nc = bacc.Bacc("TRN2", target_bir_lowering=False, debug=False)
    pcq_in = nc.dram_tensor("pcq", [B_PER, 7, N0], F32, kind="ExternalInput")
    pcf_in = nc.dram_tensor("pcf", [B_PER, 3, N0], F32, kind="ExternalInput")
    identf_in = nc.dram_tensor("identf", [128, 128], F32, kind="ExternalInput")
    g0_in = nc.dram_tensor("g0row", [1, COUT[3]], F32, kind="ExternalInput")
    win = {}
    for n in range(4):
        c = cfg(n)
        ff = c["ff"]
        fw = min(ff, 128)
        kin = 3 if n == 0 else c["cin"]
        shapes = {
            f"wtabv{n}": [kin, 16 * c["vb"]],
            f"btabv{n}": [16 * c["vb"], 1],
            f"wtabs{n}": [kin, 4],
            f"wd0_{n}": [kin, H],
            f"bd0_{n}": [H, 1],
            f"wr1x{n}": [128, c["ec"]],
            f"br1_{n}": [c["ec"], 1],
            f"wgl_{n}": [c["ec"], (c["le_off"] + 4 if c["hcv"] < 128 else 128)],
            f"wgle{n}": [c["ec"], 4],
            f"bgl_{n}": [c["hcv"], 1],
            f"wo_{n}": [c["hcv"], c["cout"]],
            f"wf1_{n}": [c["cout"], ff],
            f"wf2_{n}": [fw, (ff // fw) * c["cout"]],
            f"mu128_{n}": [128, 1],
        }
        for nm, shp in shapes.items():
            win[nm] = nc.dram_tensor(nm, shp, F32, kind="ExternalInput")
    out0 = nc.dram_tensor("out0", [B_PER, COUT[3]], F32, kind="ExternalOutput")
    dbg = {}
    if DEBUG:
        for nm, shp, dt_ in [("dbg_idxw0", [16, 1024], U16),
                             ("dbg_d2", [128, 128], F32),
                             ("dbg_ta", [1, 4096], F32),
                             ("dbg_tb", [1, 2560], F32),
                             ("dbg_msg0", [16, 1024], F32),
                             ("dbg_d0", [4, 1024], F32),
                             ("dbg_emb0", [64, 1024], F32)]:
            dbg[nm] = nc.dram_tensor(nm, shp, dt_, kind="ExternalOutput")

    with tile.TileContext(nc) as tc:
        with tc.tile_pool(name="const", bufs=1) as cpool, \
             tc.tile_pool(name="state", bufs=1) as stpool, \
             tc.tile_pool(name="big", bufs=1) as bpool, \
             tc.tile_pool(name="work", bufs=1) as wpool, \
             tc.tile_pool(name="dram", bufs=1, space="DRAM") as dpool:

            identf = cpool.tile([128, 128], F32)
            nc.sync.dma_start(identf[:], identf_in[:])
            g0row = cpool.tile([1, COUT[3]], F32)
            nc.sync.dma_start(g0row[:], g0_in[:])
            pihalf = cpool.tile([128, 1], F32)
            nc.vector.memset(pihalf[:], PI / 2.0)
            # (no ones7: the q matmul uses K=6, the ones row is unused)
            eps6 = cpool.tile([1, 1], F32)
            nc.vector.memset(eps6[:], 1e-6)
            wt = {}
            for nm, dt_ in win.items():
                t = cpool.tile(list(dt_.shape), F32, tag=nm)
                nc.sync.dma_start(t[:], dt_[:])
                wt[nm] = t

            TA = dpool.tile([1, 16384 * 32 + 4096 * 48 + 1024 * 80 + 16 * 144],
                            F32, tag="TA")
            TB = dpool.tile([1, 16384 * 20], F32, tag="TB")
            DE = dpool.tile([1, 2 * ND[0] * K], F32, tag="DE")

            for b in range(B_PER):
                pcq = stpool.tile([7, N0], F32, tag="pcq")
                nc.sync.dma_start(pcq[:], pcq_in[b])
                pcf = stpool.tile([3, N0], F32, tag="pcf")
                nc.sync.dma_start(pcf[:], pcf_in[b])

                embT = None
                for n in range(4):
                    c = cfg(n)
                    ns, nd, e_cnt = c["ns"], c["nd"], c["e"]
                    ntl, tr = c["ntiles"], c["tr"]
                    hcv, ec, vb, s0b = c["hcv"], c["ec"], c["vb"], c["s0base"]
                    wa_w, wb_w, le_off = c["wa"], c["wb"], c["le_off"]
                    dst_str = c["stride"] * 4
                    coordsT = pcq[0:3, 0:N0:c["stride"]]
                    rhs_q = pcq[0:6, 0:N0:c["stride"]]
                    TAv = TA[0:1, 0:e_cnt * wa_w].rearrange(
                        "z (e w) -> z e w", w=wa_w)
                    TBv = TB[0:1, 0:e_cnt * wb_w].rearrange(
                        "z (e w) -> z e w", w=wb_w)

                    # ===== phase S: selection =====
                    lhsq = wpool.tile([6, max(nd, 128)], F32, tag="lhsq")
                    nc.vector.memset(lhsq[:, 0:nd], -1.0)
                    if n < 3:
                        nc.vector.tensor_scalar_mul(
                            lhsq[0:3, 0:nd], pcq[0:3, 0:N0:dst_str], 2.0)
                    else:
                        nc.vector.memset(lhsq[0:3, 0:nd], 0.0)

                    idxw = wpool.tile([16, max(nd, 8)], U16, tag="idxw")
                    with tc.tile_pool(name="psS", bufs=1, space="PSUM") as psS:
                        for t in range(ntl):
                            qsb = bpool.tile([128, ns], F32, tag="qsb")
                            nh = 2 if ns > 2048 else 1
                            hw = ns // nh
                            for hh in range(nh):
                                qp = psS.tile([tr, hw], F32, tag="qpsum")
                                for cc in range(0, hw, 512):
                                    nw = min(512, hw - cc)
                                    nc.tensor.matmul(
                                        qp[:, cc:cc + nw],
                                        lhsq[:, t * 128:t * 128 + tr],
                                        rhs_q[:, hh * hw + cc:hh * hw + cc + nw],
                                        start=True, stop=True)
                                nc.any.tensor_copy(
                                    qsb[0:tr, hh * hw:(hh + 1) * hw], qp[:])
                            v8 = wpool.tile([tr, 8], F32, tag="v8")
                            idx16 = wpool.tile([tr, K], U16, tag="idx16")
                            nc.vector.max(out=v8[:], in_=qsb[0:tr, :])
                            nc.vector.max_index(out=idx16[:, 0:8], in_max=v8[:],
                                                in_values=qsb[0:tr, :])
                            nc.vector.match_replace(
                                out=qsb[0:tr, :], in_to_replace=v8[:],
                                in_values=qsb[0:tr, :], imm_value=-1e30)
                            nc.vector.max(out=v8[:], in_=qsb[0:tr, :])
                            nc.vector.max_index(out=idx16[:, 8:16], in_max=v8[:],
                                                in_values=qsb[0:tr, :])
                            idxf = wpool.tile([tr, K], F32, tag="idxf")
                            nc.vector.tensor_copy(idxf[:], idx16[:])
                            ptr = psS.tile([K, tr], F32, tag="ptr")
                            nc.tensor.transpose(ptr[:], idxf[:],
                                                identf[0:tr, 0:tr])
                            nc.vector.tensor_copy(
                                idxw[:, t * 128:t * 128 + tr], ptr[:])

                    if DEBUG and b == 0 and n == 0:
                        nc.sync.dma_start(dbg["dbg_idxw0"][:], idxw[:, 0:nd])
                    idxrep = bpool.tile([128, max(nd, 8)], U16, tag="idxrep")
                    for g in range(8):
                        nc.sync.dma_start(idxrep[16 * g:16 * g + 16, 0:nd],
                                          idxw[:, 0:nd])

                    # ===== phase G: tables + gather + TA dump =====
                    tbl = bpool.tile([128, ns], F32, tag="tbl")
                    nc.gpsimd.memset(tbl[:, :], 0.0)
                    if n == 3:
                        tbl2 = bpool.tile([128, ns], F32, tag="tbl2")
                        nc.gpsimd.memset(tbl2[:, :], 0.0)
                    x_rhs = pcf[0:3, :] if n == 0 else embT[:, 0:ns]
                    with tc.tile_pool(name="psG", bufs=1, space="PSUM") as psG:
                        for cc in range(0, ns, 2048):
                            nw = min(2048, ns - cc)
                            tp = psG.tile([16 * vb, nw], F32, tag="tpv")
                            _mm(nc, tp, wt[f"wtabv{n}"][:],
                                x_rhs[:, cc:cc + nw], nw)
                            nc.vector.tensor_scalar_add(
                                tbl[0:16 * vb, cc:cc + nw], tp[:, 0:nw],
                                wt[f"btabv{n}"][:])
                            ts_ = psG.tile([4, nw], F32, tag="tps")
                            _mm(nc, ts_, wt[f"wtabs{n}"][:],
                                x_rhs[:, cc:cc + nw], nw)
                            if n < 3:
                                nc.any.tensor_copy(tbl[s0b:s0b + 4, cc:cc + nw],
                                                   ts_[:, 0:nw])
                            else:
                                nc.any.tensor_copy(tbl2[0:4, cc:cc + nw],
                                                   ts_[:, 0:nw])
                    if n < 3:
                        nc.sync.dma_start(tbl[s0b + 4:s0b + 7, 0:ns], coordsT)
                    else:
                        nc.sync.dma_start(tbl2[4:7, 0:ns], coordsT)
                    chans = 128

                    gw = min(e_cnt, 1024)
                    gq = e_cnt // gw
                    for q in range(gq):
                        gout = bpool.tile([128, max(gw, 8)], F32, tag="gout")
                        nc.gpsimd.indirect_copy(
                            gout[:, 0:gw], tbl[:, 0:ns],
                            idxrep[:, q * (gw // 16):(q + 1) * (gw // 16)],
                            i_know_ap_gather_is_preferred=True)
                        for vblk in range(vb):
                            nc.sync.dma_start(
                                TAv[0, q * gw:(q + 1) * gw,
                                    16 * vblk:16 * (vblk + 1)]
                                .rearrange("e w -> w e"),
                                gout[16 * vblk:16 * vblk + 16, 0:gw])
                        if n < 3:
                            nc.sync.dma_start(
                                TAv[0, q * gw:(q + 1) * gw, hcv:hcv + 16]
                                .rearrange("e w -> w e"),
                                gout[s0b:s0b + 16, 0:gw])
                        else:
                            gout2 = bpool.tile([128, max(gw, 8)], F32, tag="gout2")
                            nc.gpsimd.indirect_copy(
                                gout2[:, 0:gw], tbl2[:, 0:ns],
                                idxrep[:, q * (gw // 16):(q + 1) * (gw // 16)],
                                i_know_ap_gather_is_preferred=True)
                            nc.sync.dma_start(
                                TAv[0, q * gw:(q + 1) * gw, hcv:hcv + 16]
                                .rearrange("e w -> w e"),
                                gout2[0:16, 0:gw])

                    # ===== phase D: exact d2/d/env =====
                    d2ar = wpool.tile([tr, ntl * K], F32, tag="d2ar")
                    with tc.tile_pool(name="psD", bufs=2, space="PSUM") as psD:
                        for t in range(ntl):
                            gA = wpool.tile([tr, K * wa_w], F32, tag="gA")
                            nc.sync.dma_start(
                                gA[:, :],
                                TAv[0, t * 128 * K:(t * 128 + tr) * K, :]
                                .rearrange("(d k) w -> d (k w)", k=K))
                            dstc = wpool.tile([tr, 4], F32, tag="dstc")
                            if n < 3:
                                pdc = psD.tile([tr, 4], F32, tag="pdc")
                                st0 = t * 128 * dst_str
                                dslice = pcq[0:3, st0:st0 + tr * dst_str:dst_str]
                                nc.tensor.transpose(pdc[:, 0:3], dslice,
                                                    identf[0:3, 0:3])
                                nc.vector.tensor_copy(dstc[:, 0:3], pdc[:, 0:3])
                            else:
                                nc.vector.memset(dstc[:, 0:3], 0.0)
                            gAv = gA[:, :].rearrange("p (k w) -> p k w", w=wa_w)
                            acc = wpool.tile([tr, K], F32, tag="dacc")
                            tmp = wpool.tile([tr, K], F32, tag="dtmp")
                            for ci in range(3):
                                cs = gAv[:, :, hcv + 4 + ci]
                                nc.vector.tensor_scalar_sub(tmp[:], cs,
                                                            dstc[:, ci:ci + 1])
                                if ci == 0:
                                    nc.vector.tensor_tensor(
                                        out=acc[:], in0=tmp[:], in1=tmp[:],
                                        op=ALU.mult)
                                else:
                                    nc.vector.tensor_tensor(
                                        out=tmp[:], in0=tmp[:], in1=tmp[:],
                                        op=ALU.mult)
                                    nc.vector.tensor_tensor(
                                        out=acc[:], in0=acc[:], in1=tmp[:],
                                        op=ALU.add)
                            nc.vector.tensor_scalar_add(
                                d2ar[:, t * K:(t + 1) * K], acc[:], 1e-12)
                    if DEBUG and b == 0 and n == 0:
                        nc.sync.dma_start(dbg["dbg_d2"][:], d2ar[:, :])
                    dear = wpool.tile([tr, ntl * K * 2], F32, tag="dear")
                    deav = dear[:, :].rearrange("p (e two) -> p e two", two=2)
                    nc.scalar.activation(deav[:, :, 0], d2ar[:, :], AF.Sqrt)
                    uar = wpool.tile([tr, ntl * K], F32, tag="uar")
                    nc.vector.tensor_scalar(uar[:], deav[:, :, 0],
                                            scalar1=1.0 / c["cutoff"],
                                            scalar2=1.0, op0=ALU.mult,
                                            op1=ALU.min)
                    nc.scalar.activation(uar[:], uar[:], AF.Sin,
                                         bias=pihalf[0:tr, :], scale=-PI)
                    nc.vector.tensor_scalar_add(deav[:, :, 1], uar[:], 1.0)
                    for t in range(ntl):
                        nc.sync.dma_start(
                            DE[0:1, t * 128 * K * 2:(t * 128 + tr) * K * 2],
                            dear[:, t * K * 2:(t + 1) * K * 2])

                    # ===== phase F: feature-major radial chain =====
                    QW = min(e_cnt, 4096)
                    nq = e_cnt // QW
                    qh = max(QW // 2, 8)
                    gl_m = le_off + 4 if hcv < 128 else 128
                    with tc.tile_pool(name="psF", bufs=1, space="PSUM") as psF:
                        for q in range(nq):
                            qbase = q * QW
                            db = bpool.tile([128, qh], F32, tag="db")
                            evt = bpool.tile([128, qh], F32, tag="evt")
                            for h2 in range(2):
                                lo = (qbase + h2 * (QW // 2)) * 2
                                hi = lo + QW
                                nc.sync.dma_start(
                                    db[64 * h2:64 * h2 + 64, 0:QW // 2],
                                    DE[0:1, lo:hi:2].partition_broadcast(64))
                                nc.sync.dma_start(
                                    evt[64 * h2:64 * h2 + 64, 0:QW // 2],
                                    DE[0:1, lo + 1:hi:2].partition_broadcast(64))
                            g_t = bpool.tile([128, qh], F32, tag="g_t")
                            nc.vector.tensor_scalar(
                                g_t[:], db[:], scalar1=wt[f"mu128_{n}"][:],
                                scalar2=1.0 / c["std"], op0=ALU.subtract,
                                op1=ALU.mult)
                            nc.scalar.activation(g_t[:], g_t[:], AF.Square)
                            nc.scalar.activation(g_t[:], g_t[:], AF.Exp,
                                                 scale=-0.5)
                            nc.vector.tensor_tensor(out=g_t[:], in0=g_t[:],
                                                    in1=evt[:], op=ALU.mult)
                            cw = min(1024, QW // 2)
                            for h2 in range(2):
                                for cc in range(0, QW // 2, cw):
                                    ebase = qbase + h2 * (QW // 2) + cc
                                    pe1 = psF.tile([ec, cw], F32, tag="pe1")
                                    _mm(nc, pe1,
                                        wt[f"wr1x{n}"][64 * h2:64 * h2 + 64, :],
                                        g_t[64 * h2:64 * h2 + 64, cc:cc + cw],
                                        cw)
                                    s_sb = wpool.tile([ec, cw], F32, tag="s_sb")
                                    _silu(nc, wpool, s_sb[:], pe1[:],
                                          wt[f"br1_{n}"][:], ec, cw)
                                    pgl = psF.tile([gl_m, cw], F32, tag="pgl")
                                    _mm(nc, pgl, wt[f"wgl_{n}"][:], s_sb[:], cw)
                                    gl_sb = wpool.tile([gl_m, cw], F32,
                                                       tag="gl_sb")
                                    nc.scalar.activation(
                                        gl_sb[0:hcv, :], pgl[0:hcv, :],
                                        AF.Sigmoid, bias=wt[f"bgl_{n}"][:])
                                    nc.sync.dma_start(
                                        TBv[0, ebase:ebase + cw, 0:hcv]
                                        .rearrange("e w -> w e"),
                                        gl_sb[0:hcv, :])
                                    if hcv < 128:
                                        nc.any.tensor_copy(
                                            gl_sb[le_off:le_off + 4, :],
                                            pgl[le_off:le_off + 4, :])
                                        nc.sync.dma_start(
                                            TBv[0, ebase:ebase + cw,
                                                hcv:hcv + 4]
                                            .rearrange("e w -> w e"),
                                            gl_sb[le_off:le_off + 4, :])
                                    else:
                                        ple = psF.tile([4, cw], F32, tag="ple")
                                        _mm(nc, ple, wt[f"wgle{n}"][:], s_sb[:],
                                            cw)
                                        le_sb = wpool.tile([4, cw], F32,
                                                           tag="le_sb")
                                        nc.any.tensor_copy(le_sb[:], ple[:])
                                        nc.sync.dma_start(
                                            TBv[0, ebase:ebase + cw,
                                                hcv:hcv + 4]
                                            .rearrange("e w -> w e"), le_sb[:])

                    if DEBUG and b == 0 and n == 0:
                        nc.sync.dma_start(dbg["dbg_ta"][:], TA[0:1, 0:4096])
                        nc.sync.dma_start(dbg["dbg_tb"][:], TB[0:1, 0:2560])
                    # ===== d0l feature-major [H, nd] =====
                    d0fm = wpool.tile([H, max(nd, 8)], F32, tag="d0fm")
                    with tc.tile_pool(name="psE", bufs=1, space="PSUM") as psE:
                        if n < 3:
                            pd0 = psE.tile([H, nd], F32, tag="pd0")
                            xdst = (pcf[0:3, 0:N0:dst_str] if n == 0
                                    else embT[:, 0:ns:4])
                            _mm(nc, pd0, wt[f"wd0_{n}"][:], xdst, nd)
                            nc.vector.tensor_scalar_add(d0fm[:, 0:nd], pd0[:],
                                                        wt[f"bd0_{n}"][:])
                        else:
                            nc.vector.tensor_copy(d0fm[:, 0:1],
                                                  wt[f"bd0_{n}"][:])

                    # ===== phase M: softmax + message =====
                    msgT = bpool.tile([hcv, max(nd, 8)], F32, tag="msgT")
                    with tc.tile_pool(name="psM", bufs=2, space="PSUM") as psM:
                        for t in range(ntl):
                            gB = wpool.tile([tr, K * wb_w], F32, tag="gB")
                            nc.sync.dma_start(
                                gB[:, :],
                                TBv[0, t * 128 * K:(t * 128 + tr) * K, :]
                                .rearrange("(d k) w -> d (k w)", k=K))
                            gBv = gB[:, :].rearrange("p (k w) -> p k w", w=wb_w)
                            gA = wpool.tile([tr, K * wa_w], F32, tag="gA")
                            nc.sync.dma_start(
                                gA[:, :],
                                TAv[0, t * 128 * K:(t * 128 + tr) * K, :]
                                .rearrange("(d k) w -> d (k w)", k=K))
                            gAv = gA[:, :].rearrange("p (k w) -> p k w", w=wa_w)
                            d0dm = wpool.tile([tr, H], F32, tag="d0dm")
                            pd0t = psM.tile([tr, H], F32, tag="pd0t")
                            nc.tensor.transpose(pd0t[:],
                                                d0fm[:, t * 128:t * 128 + tr],
                                                identf[0:H, 0:H])
                            nc.vector.tensor_copy(d0dm[:], pd0t[:])
                            lg = wpool.tile([tr, K * H], F32, tag="lg")
                            lgv = lg[:, :].rearrange("p (k h) -> p k h", h=H)
                            nc.vector.tensor_tensor(
                                out=lgv[:, :, :], in0=gBv[:, :, hcv:hcv + 4],
                                in1=gAv[:, :, hcv:hcv + 4], op=ALU.add)
                            d0b = d0dm[:, :].to_broadcast([tr, H, K]) \
                                .rearrange("p h k -> p k h")
                            nc.vector.tensor_tensor(out=lgv[:, :, :],
                                                    in0=lgv[:, :, :], in1=d0b,
                                                    op=ALU.add)
                            mx = wpool.tile([tr, H], F32, tag="mx")
                            lgkv = lg[:, :].rearrange("p (k h) -> p h k", h=H)
                            nc.vector.tensor_reduce(mx[:], lgkv,
                                                    mybir.AxisListType.X,
                                                    ALU.max)
                            mxb = mx[:, :].to_broadcast([tr, H, K]) \
                                .rearrange("p h k -> p k h")
                            nc.vector.tensor_tensor(out=lgv[:, :, :],
                                                    in0=lgv[:, :, :], in1=mxb,
                                                    op=ALU.subtract)
                            nc.scalar.activation(lg[:, :], lg[:, :], AF.Exp)
                            ssum = wpool.tile([tr, H], F32, tag="ssum")
                            nc.vector.tensor_reduce(ssum[:], lgkv,
                                                    mybir.AxisListType.X,
                                                    ALU.add)
                            rec = wpool.tile([tr, H], F32, tag="rec")
                            nc.vector.reciprocal(rec[:], ssum[:])
                            P = wpool.tile([tr, K * hcv], F32, tag="P")
                            Pv = P[:, :].rearrange("p (k c) -> p k c", c=hcv)
                            nc.vector.tensor_tensor(out=Pv[:, :, :],
                                                    in0=gAv[:, :, 0:hcv],
                                                    in1=gBv[:, :, 0:hcv],
                                                    op=ALU.mult)
                            lgb = lg[:, :].rearrange("p (k h) -> p k h", h=H) \
                                .to_broadcast([tr, K, H, c["cv"]])
                            Pv4 = P[:, :].rearrange("p (k h v) -> p k h v",
                                                    h=H, v=c["cv"])
                            nc.vector.tensor_tensor(out=Pv4[:, :, :, :],
                                                    in0=Pv4[:, :, :, :],
                                                    in1=lgb, op=ALU.mult)
                            msum = wpool.tile([tr, hcv], F32, tag="msum")
                            Pkv = P[:, :].rearrange("p (k c) -> p c k", c=hcv)
                            nc.vector.tensor_reduce(msum[:], Pkv,
                                                    mybir.AxisListType.X,
                                                    ALU.add)
                            recb = rec[:, :].to_broadcast([tr, H, c["cv"]])
                            msv = msum[:, :].rearrange("p (h v) -> p h v",
                                                       v=c["cv"])
                            nc.vector.tensor_tensor(out=msv[:, :, :],
                                                    in0=msv[:, :, :], in1=recb,
                                                    op=ALU.mult)
                            pmt = psM.tile([hcv, tr], F32, tag="pmt")
                            nc.tensor.transpose(pmt[:], msum[:],
                                                identf[0:tr, 0:tr])
                            nc.vector.tensor_copy(
                                msgT[:, t * 128:t * 128 + tr], pmt[:])

                    if DEBUG and b == 0 and n == 0:
                        nc.sync.dma_start(dbg["dbg_msg0"][:], msgT[:, 0:nd])
                        nc.sync.dma_start(dbg["dbg_d0"][:], d0fm[:, 0:nd])
                    # ===== phase O: out = msg@wo + FFN =====
                    with tc.tile_pool(name="psO", bufs=1, space="PSUM") as psO:
                        po1 = psO.tile([c["cout"], max(nd, 8)], F32, tag="po1")
                        _mm(nc, po1, wt[f"wo_{n}"][:], msgT[:, 0:nd], nd)
                        o1 = wpool.tile([c["cout"], max(nd, 8)], F32, tag="o1")
                        nc.vector.tensor_copy(o1[:, 0:nd], po1[:, 0:nd])
                        fw = min(c["ff"], 128)
                        nf = c["ff"] // fw
                        pf2 = psO.tile([c["cout"], max(nd, 8)], F32, tag="pf2")
                        for fi in range(nf):
                            pf1 = psO.tile([fw, max(nd, 8)], F32, tag="pf1")
                            _mm(nc, pf1,
                                wt[f"wf1_{n}"][:, fi * fw:(fi + 1) * fw],
                                o1[:, 0:nd], nd)
                            sf = wpool.tile([fw, max(nd, 8)], F32, tag="sf")
                            _silu(nc, wpool, sf[:, 0:nd], pf1[:, 0:nd], None,
                                  fw, nd)
                            _mm(nc, pf2,
                                wt[f"wf2_{n}"][:,
                                               fi * c["cout"]:(fi + 1) * c["cout"]],
                                sf[:, 0:nd], nd, start=(fi == 0),
                                stop=(fi == nf - 1))
                        embT_new = stpool.tile([c["cout"], max(nd, 8)], F32,
                                               tag="embT")
                        nc.vector.tensor_tensor(out=embT_new[:, 0:nd],
                                                in0=o1[:, 0:nd],
                                                in1=pf2[:, 0:nd], op=ALU.add)
                        embT = embT_new
                    if DEBUG and b == 0 and n == 0:
                        nc.sync.dma_start(dbg["dbg_emb0"][:], embT[:, 0:nd])

                # ===== final RMS =====
                with tc.tile_pool(name="psR", bufs=1, space="PSUM") as psR:
                    pfin = psR.tile([1, COUT[3]], F32, tag="pfin")
                    nc.tensor.transpose(pfin[:], embT[:, 0:1], identf[0:COUT[3], 0:COUT[3]])
                    r0 = wpool.tile([1, COUT[3]], F32, tag="r0")
                    nc.vector.tensor_copy(r0[:], pfin[:])
                    sq = wpool.tile([1, COUT[3]], F32, tag="sqf")
                    nc.vector.tensor_tensor(out=sq[:], in0=r0[:], in1=r0[:],
                                            op=ALU.mult)
                    ssf = wpool.tile([1, 8], F32, tag="ssf")
                    nc.vector.tensor_reduce(ssf[:, 0:1], sq[:],
                                            mybir.AxisListType.X, ALU.add)
                    nc.scalar.activation(ssf[:, 0:1], ssf[:, 0:1], AF.Sqrt,
                                         scale=1.0 / COUT[3], bias=eps6[:])
                    nc.vector.reciprocal(ssf[:, 0:1], ssf[:, 0:1])
                    nc.vector.tensor_scalar_mul(r0[:], r0[:], ssf[0:1, 0:1])
                    nc.vector.tensor_tensor(out=r0[:], in0=r0[:], in1=g0row[:],
                                            op=ALU.mult)
                    nc.sync.dma_start(out0[b:b + 1, :], r0[:])
    nc.compile()
    return nc


_NC_CACHE = None


def _prep_params(params):
    W0 = np.asarray(params["W0"], np.float32)
    b0 = np.asarray(params["b0"], np.float32)
    g_norm = np.asarray(params["g_norm"], np.float32)
    scales = params["scales"]
    out = {}
    for n in range(4):
        c = cfg(n)
        sp = {k: np.asarray(v, np.float32) for k, v in scales[n].items()}
        hcv, ec, cin = c["hcv"], c["ec"], c["cin"]
        if n == 0:
            wv_eff = W0 @ sp["wv"]
            was_eff = W0 @ sp["wa"][:cin]
            wad_eff = W0 @ sp["wa"][cin:2 * cin]
            bv = b0 @ sp["wv"]
            bs = b0 @ sp["wa"][:cin]
            bd = b0 @ sp["wa"][cin:2 * cin]
        else:
            wv_eff = sp["wv"]
            was_eff = sp["wa"][:cin]
            wad_eff = sp["wa"][cin:2 * cin]
            bv = np.zeros(hcv, np.float32)
            bs = np.zeros(H, np.float32)
            bd = np.zeros(H, np.float32)
        wae = sp["wa"][2 * cin:]
        wr2, br2 = sp["wr2"], sp["br2"]
        out[f"wtabv{n}"] = np.ascontiguousarray(wv_eff)
        out[f"btabv{n}"] = bv.reshape(-1, 1)
        out[f"wtabs{n}"] = np.ascontiguousarray(was_eff)
        out[f"wd0_{n}"] = np.ascontiguousarray(wad_eff)
        out[f"bd0_{n}"] = (bd + bs + br2 @ wae).reshape(-1, 1)
        wr1h = (0.5 * sp["wr1"]).astype(np.float32)
        out[f"wr1x{n}"] = np.concatenate([wr1h, wr1h], 0)
        out[f"br1_{n}"] = sp["br1"].reshape(-1, 1)
        wg_f = wr2 @ sp["wg"]
        wae_f = wr2 @ wae
        if hcv < 128:
            le_off = c["le_off"]
            wgl = np.zeros((ec, le_off + 4), np.float32)
            wgl[:, 0:hcv] = wg_f
            wgl[:, le_off:le_off + 4] = wae_f
            out[f"wgl_{n}"] = wgl
            out[f"wgle{n}"] = np.zeros((ec, 4), np.float32)
        else:
            out[f"wgl_{n}"] = wg_f
            out[f"wgle{n}"] = wae_f
        out[f"bgl_{n}"] = (sp["bg"] + br2 @ sp["wg"]).reshape(-1, 1)
        out[f"wo_{n}"] = sp["wo"]
        out[f"wf1_{n}"] = sp["wf1"]
        fw = min(c["ff"], 128)
        nf = c["ff"] // fw
        out[f"wf2_{n}"] = np.concatenate(
            [sp["wf2"][fi * fw:(fi + 1) * fw] for fi in range(nf)], axis=1)
        mu = np.linspace(0.0, c["cutoff"], NB).astype(np.float32)
        out[f"mu128_{n}"] = np.concatenate([mu, mu]).reshape(-1, 1)
    out["g0row"] = np.ascontiguousarray(g_norm[0].reshape(1, -1))
    out["identf"] = np.eye(128, dtype=np.float32)
    return out


def kernel(point_cloud, robot0_eef_quat, robot0_gripper_qpos, robot0_eef_pos,
           params):
    global _NC_CACHE
    pc = np.asarray(point_cloud, np.float32)
    B = pc.shape[0]
    wdict = _prep_params(params)
    if _NC_CACHE is None:
        _NC_CACHE = build_nc()
    nc = _NC_CACHE

    pcT = np.ascontiguousarray(pc.transpose(0, 2, 1))
    coords = pcT[:, 0:3]
    feats = np.ascontiguousarray(pcT[:, 3:6])
    sq = coords * coords
    ones = np.ones((B, 1, N0), np.float32)
    pcq_all = np.ascontiguousarray(np.concatenate([coords, sq, ones], 1))

    in_maps = []
    for core in range(N_CORES):
        sl = slice(core * B_PER, (core + 1) * B_PER)
        m = dict(pcq=np.ascontiguousarray(pcq_all[sl]),
                 pcf=np.ascontiguousarray(feats[sl]))
        m.update(wdict)
        in_maps.append(m)
    res = run_bass_kernel_spmd(nc, in_maps, list(range(N_CORES)), trace=TRACE)
    global LAST_RESULTS
    LAST_RESULTS = res
    out = np.zeros((B, 9, COUT[3]), np.float32)
    for core in range(N_CORES):
        out[core * B_PER:(core + 1) * B_PER, 0, :] = res.results[core]["out0"]
    return out
